# revision 4
# baseline (speedup 1.0000x reference)
"""StyleGAN2-style modulated 3x3 conv (B=8, Ci=Co=512, H=W=32) on 8 TRN2 NeuronCores.

Sharding: data-parallel over batch, one sample per core (embarrassingly
parallel, no collectives). Per core the conv is computed as 9 shifted
matmuls over a zero-padded 34x34 image held in SBUF, contracting over
Ci in 128-chunks with fp32 accumulation in PSUM; compute dtype bf16.

Math (per sample b, with s = (Ci*K*K)**-0.5 folded out of both the conv
and the demod norm so the weights can be used unscaled):
  conv = conv2d(x * y_s, weight)                     # raw, no s
  xs2[o] = sum_i y_s[i]^2 * w2[i,o],  w2 = sum_k weight[o,i,k]^2
  out = conv / sqrt(xs2 + 1e-8 * Ci * K * K) + bias

Host-side prep is layout only plus the input-independent w2 fold
(a pure weight transform, shipped as a 10th "k-slot" of the weight
tensor); all input-dependent math runs on device.

Schedule notes (from trace analysis of the 84us version):
- The MM stream itself runs at the N=512 bf16 roofline (~216ns/MM), so
  the wins are all at the edges: start the stream sooner, keep HAM warm
  from the first useful matmul, and shrink the tail.
- DMA order is priority order *per queue*: sync carries yb then x
  (head-first), scalar carries weights in consumption order with the
  first (j0,q0) tile split in half so the first conv MMs start sooner.
- The end-of-kernel teardown emits per-engine semaphore-wait trains
  proportional to the number of DMA transfers/semaphores, so transfers
  are batched (weights: 8 transfers, outputs: per-co-quarter).
- Demod is split per co-half (q01 / q23) so early epilogues only wait
  on weights they already needed for the conv itself.
"""

import numpy as np
import ml_dtypes

import concourse.mybir as mybir
from concourse import bacc
from concourse.tile import TileContext
from concourse.bass_utils import run_bass_kernel_spmd

B = 8
CI = 512
CO = 512
H = W = 32
KK = 9  # 3x3
NCI = CI // 128
NCO = CO // 128
HWPAD = 34
XSPLIT = 17  # first x row-chunk: rows 0..16 (all that conv half 0 needs)
EPS_EFF = 1e-8 * CI * KK  # demod eps compensated for unscaled weights

F32 = mybir.dt.float32
BF16 = mybir.dt.bfloat16
AF = mybir.ActivationFunctionType


def build_nc():
    nc = bacc.Bacc("TRN2", target_bir_lowering=False, debug=False)

    x_ext = nc.declare_dram_parameter("x", [NCI, 128, H, W], BF16, isOutput=False)
    # cols 0..3 = y_s per ci-tile, cols 4..7 = bias per co-tile
    yb_ext = nc.declare_dram_parameter("yb", [128, 2 * NCI], F32, isOutput=False)
    # [jco, ci_p, jci, k(9)+w2(1), co_c] bf16 (co-quarter-major so each
    # quarter is one contiguous slab)
    wt_ext = nc.declare_dram_parameter(
        "wt", [NCO, 128, NCI, KK + 1, 128], BF16, isOutput=False
    )
    out_ext = nc.declare_dram_parameter("out", [NCO, 128, H * W], F32, isOutput=True)

    with TileContext(nc) as tc:
        with (
            tc.tile_pool(name="singles", bufs=1) as singles,
            tc.tile_pool(name="wts", bufs=1) as wts,
            tc.tile_pool(name="pads", bufs=1) as pads,
            tc.tile_pool(name="xin", bufs=1) as xin,
            tc.tile_pool(name="outs", bufs=3) as outs,
            tc.tile_pool(name="cps", bufs=5, space="PSUM") as cps,
            tc.tile_pool(name="d01", bufs=1, space="PSUM") as d01,
            tc.tile_pool(name="d23", bufs=1, space="PSUM") as d23,
            tc.tile_pool(name="wps", bufs=1, space="PSUM") as wps,
        ):
            # ---- input DMAs, priority order per queue ----
            # sync queue: yb (tiny, unblocks modulation+ys2), then x head
            # first. scalar queue: weights in consumption order; the first
            # (q0) slab is split so the first conv MMs only wait for ~1/8
            # of a quarter.
            xt_sb = [
                xin.tile([128, H, W], BF16, tag=f"x{j}", name=f"xt{j}")
                for j in range(NCI)
            ]
            yb_sb = singles.tile([128, 2 * NCI], F32)
            nc.sync.dma_start(out=yb_sb, in_=yb_ext[:, :])
            nc.sync.dma_start(out=xt_sb[0][:, 0:XSPLIT, :], in_=x_ext[0][:, 0:XSPLIT, :])
            nc.sync.dma_start(out=xt_sb[0][:, XSPLIT:H, :], in_=x_ext[0][:, XSPLIT:H, :])
            for j in range(1, NCI):
                nc.sync.dma_start(out=xt_sb[j], in_=x_ext[j])

            # weight tiles: one [128, jci, 10, 128] tile per co-quarter
            wq_sb = [
                wts.tile([128, NCI, KK + 1, 128], BF16, tag=f"wq{q}", name=f"wq{q}")
                for q in range(NCO)
            ]
            nc.scalar.dma_start(out=wq_sb[0][:, 0, 0:5, :], in_=wt_ext[0][:, 0, 0:5, :])
            nc.scalar.dma_start(out=wq_sb[0][:, 0, 5:10, :], in_=wt_ext[0][:, 0, 5:10, :])
            nc.scalar.dma_start(out=wq_sb[0][:, 1:4], in_=wt_ext[0][:, 1:4])
            nc.scalar.dma_start(out=wq_sb[1][:, 0:2], in_=wt_ext[1][:, 0:2])
            nc.scalar.dma_start(out=wq_sb[1][:, 2:4], in_=wt_ext[1][:, 2:4])
            nc.scalar.dma_start(out=wq_sb[2], in_=wt_ext[2])
            nc.scalar.dma_start(out=wq_sb[3], in_=wt_ext[3])

            def wt_slice(j, jo, k):
                return wq_sb[jo][:, j, k, :]

            # ---- PE warm-up: throwaway matmuls on memset data so the HAM
            # clock gate releases while the first input DMAs are in flight;
            # sized to end roughly when pad0/weights land (~3us) ----
            warm_lhs = singles.tile([128, 1], BF16)
            nc.vector.memset(warm_lhs, 1.0)
            warm_rhs = singles.tile([128, 512], BF16)
            nc.vector.memset(warm_rhs, 0.5)
            warm_ps = wps.tile([1, 512], F32)
            N_WARM = 6
            for i in range(N_WARM):
                nc.tensor.matmul(
                    out=warm_ps,
                    lhsT=warm_lhs,
                    rhs=warm_rhs,
                    start=(i == 0),
                    stop=(i == N_WARM - 1),
                )

            eps_sb = singles.tile([128, 1], F32)
            nc.vector.memset(eps_sb, EPS_EFF)

            # ---- zero-padded modulated input (bf16), border-only memsets ----
            pad_sb = []
            for j in range(NCI):
                p = pads.tile([128, HWPAD, HWPAD], BF16, tag=f"pad{j}")
                nc.gpsimd.memset(p[:, 0, :], 0.0)
                nc.gpsimd.memset(p[:, HWPAD - 1, :], 0.0)
                nc.gpsimd.memset(p[:, 1 : HWPAD - 1, 0], 0.0)
                nc.gpsimd.memset(p[:, 1 : HWPAD - 1, HWPAD - 1], 0.0)
                pad_sb.append(p)

            def mod(j, r0, r1):
                nc.vector.tensor_scalar(
                    out=pad_sb[j][:, 1 + r0 : 1 + r1, 1 : W + 1],
                    in0=xt_sb[j][:, r0:r1, :],
                    scalar1=yb_sb[:, j : j + 1],
                    scalar2=None,
                    op0=mybir.AluOpType.mult,
                )

            mod(0, 0, XSPLIT)
            # ys^2 in bf16 for the demod matmuls; only needs yb
            ys2_sb = singles.tile([128, NCI], BF16)
            nc.vector.tensor_mul(ys2_sb, yb_sb[:, 0:NCI], yb_sb[:, 0:NCI])
            mod(0, XSPLIT, H)
            for j in range(1, NCI):
                mod(j, 0, H)

            rs_sb = singles.tile([128, NCO], F32)

            def conv_mms(jo, half):
                ps = cps.tile([128, 512], F32, tag="ps")
                h0 = half * 16
                idx = 0
                for j in range(NCI):
                    for k in range(KK):
                        kh, kw = divmod(k, 3)
                        rhs = pad_sb[j][:, kh + h0 : kh + h0 + 16, kw : kw + W]
                        nc.tensor.matmul(
                            out=ps,
                            lhsT=wt_slice(j, jo, k),
                            rhs=rhs,
                            start=(idx == 0),
                            stop=(idx == KK * NCI - 1),
                        )
                        idx += 1
                return ps

            def demod_pair(ps, q0):
                # xs2[:, q] for q in (q0, q0+1), then rs = 1/sqrt(xs2+eps)
                for q in (q0, q0 + 1):
                    for j in range(NCI):
                        nc.tensor.matmul(
                            out=ps[:, q - q0 : q - q0 + 1],
                            lhsT=wt_slice(j, q, KK),
                            rhs=ys2_sb[:, j : j + 1],
                            start=(j == 0),
                            stop=(j == NCI - 1),
                        )

            def demod_finish(ps, q0):
                nc.scalar.activation(
                    out=rs_sb[:, q0 : q0 + 2], in_=ps, func=AF.Sqrt, bias=eps_sb
                )
                nc.vector.reciprocal(
                    out=rs_sb[:, q0 : q0 + 2], in_=rs_sb[:, q0 : q0 + 2]
                )

            def epilogue(ps, out_ap, jo, c0, c1):
                # out_ap = conv/rs + bias for columns [c0:c1) of this psum
                nc.scalar.activation(
                    out=out_ap,
                    in_=ps[:, c0:c1],
                    func=AF.Identity,
                    bias=yb_sb[:, NCI + jo : NCI + jo + 1],
                    scale=rs_sb[:, jo : jo + 1],
                )

            xs01_ps = d01.tile([128, 2], F32)
            xs23_ps = d23.tile([128, 2], F32)

            # ---- emission order IS dataflow order under Tile ----
            # PE stream: conv jo0 -> demod01 -> conv jo1 -> demod23 ->
            # conv jo2, jo3 (continuous). ACT stream: weight-DMA triggers,
            # sqrt01, epilogues jo0, sqrt23, epilogues jo1..jo3.
            ps00 = conv_mms(0, 0)
            ps01 = conv_mms(0, 1)
            demod_pair(xs01_ps, 0)
            demod_finish(xs01_ps, 0)
            ot0 = outs.tile([128, 2 * 512], F32, tag="ot")
            epilogue(ps00, ot0[:, 0:512], 0, 0, 512)
            epilogue(ps01, ot0[:, 512:1024], 0, 0, 512)
            nc.sync.dma_start(out=out_ext[0], in_=ot0)

            ps10 = conv_mms(1, 0)
            ps11 = conv_mms(1, 1)
            demod_pair(xs23_ps, 2)
            demod_finish(xs23_ps, 2)
            ot1 = outs.tile([128, 2 * 512], F32, tag="ot")
            epilogue(ps10, ot1[:, 0:512], 1, 0, 512)
            epilogue(ps11, ot1[:, 512:1024], 1, 0, 512)
            nc.sync.dma_start(out=out_ext[1], in_=ot1)

            ps20 = conv_mms(2, 0)
            ps21 = conv_mms(2, 1)
            ot2 = outs.tile([128, 2 * 512], F32, tag="ot")
            epilogue(ps20, ot2[:, 0:512], 2, 0, 512)
            epilogue(ps21, ot2[:, 512:1024], 2, 0, 512)
            nc.sync.dma_start(out=out_ext[2], in_=ot2)

            ps30 = conv_mms(3, 0)
            ot3a = outs.tile([128, 512], F32, tag="o3a")
            epilogue(ps30, ot3a, 3, 0, 512)
            nc.sync.dma_start(out=out_ext[3, :, 0:512], in_=ot3a)
            ps31 = conv_mms(3, 1)
            # final tile in two strips so the terminal DMA is small and the
            # second strip's transfer overlaps the first's
            ot3b = outs.tile([128, 256], F32, tag="o3b")
            ot3c = outs.tile([128, 256], F32, tag="o3c")
            epilogue(ps31, ot3b, 3, 0, 256)
            nc.gpsimd.dma_start(out=out_ext[3, :, 512:768], in_=ot3b)
            epilogue(ps31, ot3c, 3, 256, 512)
            nc.sync.dma_start(out=out_ext[3, :, 768:1024], in_=ot3c)

            # keep the warm-up matmuls live (cheap PSUM read at the end)
            warm_sink = singles.tile([1, 1], F32)
            nc.vector.tensor_copy(out=warm_sink, in_=warm_ps[0:1, 0:1])
    nc.compile()
    return nc


_NC_CACHE = None


def _get_nc():
    global _NC_CACHE
    if _NC_CACHE is None:
        _NC_CACHE = build_nc()
    return _NC_CACHE


def _prep_inputs(x, y_s, weight, bias):
    # [co, ci, kh, kw] -> [k, ci, co]; append w2 = sum_k wt^2 as slot 9;
    # then tile to [jco, ci_p, jci, 10, co_c] bf16 contiguous.
    wt9 = weight.transpose(2, 3, 1, 0).reshape(KK, CI, CO)
    w2 = (wt9.astype(np.float64) ** 2).sum(axis=0).astype(np.float32)
    full = np.concatenate([wt9, w2[None]], axis=0)  # [10, ci, co]
    wtq = np.ascontiguousarray(
        full.reshape(KK + 1, NCI, 128, NCO, 128).transpose(3, 2, 1, 0, 4)
    ).astype(ml_dtypes.bfloat16)
    in_maps = []
    for b in range(B):
        yb = np.empty((128, 2 * NCI), np.float32)
        yb[:, :NCI] = y_s[b].reshape(NCI, 128).T
        yb[:, NCI:] = bias.reshape(NCO, 128).T
        in_maps.append(
            {
                "x": np.ascontiguousarray(x[b].reshape(NCI, 128, H, W)).astype(
                    ml_dtypes.bfloat16
                ),
                "yb": yb,
                "wt": wtq,
            }
        )
    return in_maps


def _install_trace_support():
    """Dev-only: register the axon NTFF profiling hook + disable the
    remote artifact upload so trace=True works in this container."""
    import sys
    import types

    import concourse.bass_utils as bu

    bu.upload_artifacts = lambda tmpdir: "local://" + str(tmpdir)
    if "antenv.axon_hooks" in sys.modules:
        return
    try:
        from trn_agent_boot.trn_boot import _ntff_profile_via_ctypes

        hook = _ntff_profile_via_ctypes("/opt/axon/libaxon_pjrt.so")
    except Exception:
        return
    mod = types.ModuleType("antenv.axon_hooks")
    mod.get_axon_ntff_profile_hook = lambda: hook
    mod.set_axon_ntff_profile_hook = lambda h: None
    sys.modules["antenv.axon_hooks"] = mod


def run(x, y_s, weight, bias, trace=False, tmpdir=None):
    nc = _get_nc()
    if trace:
        _install_trace_support()
    in_maps = _prep_inputs(x, y_s, weight, bias)
    res = run_bass_kernel_spmd(
        nc, in_maps, core_ids=list(range(B)), trace=trace, tmpdir=tmpdir
    )
    out = np.stack(
        [res.results[b]["out"].reshape(CO, H, W) for b in range(B)]
    ).astype(np.float32)
    return out, res


def kernel(x, y_s, weight, bias):
    out, _ = run(
        np.asarray(x, dtype=np.float32),
        np.asarray(y_s, dtype=np.float32),
        np.asarray(weight, dtype=np.float32),
        np.asarray(bias, dtype=np.float32),
    )
    return out


# revision 5
# speedup vs baseline: 1.0692x; 1.0692x over previous
"""StyleGAN2-style modulated 3x3 conv (B=8, Ci=Co=512, H=W=32) on 8 TRN2 NeuronCores.

Sharding: data-parallel over batch, one sample per core (embarrassingly
parallel, no collectives). Per core the conv is computed as 9 shifted
matmuls over a zero-padded 34x34 image held in SBUF, contracting over
Ci in 128-chunks with fp32 accumulation in PSUM; compute dtype bf16.

Math (per sample b, with s = (Ci*K*K)**-0.5 folded out of both the conv
and the demod norm so the weights can be used unscaled):
  conv = conv2d(x * y_s, weight)                     # raw, no s
  xs2[o] = sum_i y_s[i]^2 * w2[i,o],  w2 = sum_k weight[o,i,k]^2
  out = conv / sqrt(xs2 + 1e-8 * Ci * K * K) + bias

Host-side prep is layout only plus the input-independent w2 fold
(a pure weight transform, shipped as a 10th "k-slot" of the weight
tensor); all input-dependent math runs on device.

Schedule notes (from trace analysis of the 84us version):
- The MM stream itself runs at the N=512 bf16 roofline (~216ns/MM), so
  the wins are all at the edges: start the stream sooner, keep HAM warm
  from the first useful matmul, and shrink the tail.
- DMA order is priority order *per queue*: sync carries yb then x
  (head-first), scalar carries weights in consumption order with the
  first (j0,q0) tile split in half so the first conv MMs start sooner.
- The end-of-kernel teardown emits per-engine semaphore-wait trains
  proportional to the number of DMA transfers/semaphores, so transfers
  are batched (weights: 8 transfers, outputs: per-co-quarter).
- Demod is split per co-half (q01 / q23) so early epilogues only wait
  on weights they already needed for the conv itself.
"""

import numpy as np
import ml_dtypes

import concourse.mybir as mybir
from concourse import bacc
from concourse.tile import TileContext
from concourse.bass_utils import run_bass_kernel_spmd

B = 8
CI = 512
CO = 512
H = W = 32
KK = 9  # 3x3
NCI = CI // 128
NCO = CO // 128
HWPAD = 34
XSPLIT = 17  # first x row-chunk: rows 0..16 (all that conv half 0 needs)
EPS_EFF = 1e-8 * CI * KK  # demod eps compensated for unscaled weights

F32 = mybir.dt.float32
BF16 = mybir.dt.bfloat16
AF = mybir.ActivationFunctionType


def build_nc():
    nc = bacc.Bacc("TRN2", target_bir_lowering=False, debug=False)

    x_ext = nc.declare_dram_parameter("x", [NCI, 128, H, W], BF16, isOutput=False)
    # cols 0..3 = y_s per ci-tile, cols 4..7 = bias per co-tile
    yb_ext = nc.declare_dram_parameter("yb", [128, 2 * NCI], F32, isOutput=False)
    # [jco, ci_p, jci, k(9)+w2(1), co_c] bf16 (co-quarter-major so each
    # quarter is one contiguous slab)
    wt_ext = nc.declare_dram_parameter(
        "wt", [NCO, 128, NCI, KK + 1, 128], BF16, isOutput=False
    )
    out_ext = nc.declare_dram_parameter("out", [NCO, 128, H * W], F32, isOutput=True)

    with TileContext(nc) as tc:
        with (
            tc.tile_pool(name="singles", bufs=1) as singles,
            tc.tile_pool(name="wts", bufs=1) as wts,
            tc.tile_pool(name="pads", bufs=1) as pads,
            tc.tile_pool(name="xin", bufs=1) as xin,
            tc.tile_pool(name="outs", bufs=3) as outs,
            tc.tile_pool(name="cps", bufs=5, space="PSUM") as cps,
            tc.tile_pool(name="d01", bufs=1, space="PSUM") as d01,
            tc.tile_pool(name="d23", bufs=1, space="PSUM") as d23,
            tc.tile_pool(name="wps", bufs=1, space="PSUM") as wps,
        ):
            # ---- input DMAs, priority order per queue ----
            # sync queue: yb (tiny, unblocks modulation+ys2), then x head
            # first. scalar queue: weights in consumption order; the first
            # (q0) slab is split so the first conv MMs only wait for ~1/8
            # of a quarter.
            xt_sb = [
                xin.tile([128, H, W], BF16, tag=f"x{j}", name=f"xt{j}")
                for j in range(NCI)
            ]
            yb_sb = singles.tile([128, 2 * NCI], F32)
            nc.sync.dma_start(out=xt_sb[0][:, 0:XSPLIT, :], in_=x_ext[0][:, 0:XSPLIT, :])
            nc.sync.dma_start(out=xt_sb[0][:, XSPLIT:H, :], in_=x_ext[0][:, XSPLIT:H, :])
            for j in range(1, NCI):
                nc.sync.dma_start(out=xt_sb[j], in_=x_ext[j])

            # weight tiles: one [128, jci, 10, 128] tile per co-quarter.
            # yb rides first on the scalar queue (tiny, unblocks modulation);
            # weight chunks follow in PE consumption order, sized so each
            # lands comfortably before its first matmul.
            wq_sb = [
                wts.tile([128, NCI, KK + 1, 128], BF16, tag=f"wq{q}", name=f"wq{q}")
                for q in range(NCO)
            ]
            nc.scalar.dma_start(out=yb_sb, in_=yb_ext[:, :])
            nc.scalar.dma_start(out=wq_sb[0][:, 0, 0:5, :], in_=wt_ext[0][:, 0, 0:5, :])
            nc.scalar.dma_start(out=wq_sb[0][:, 0, 5:10, :], in_=wt_ext[0][:, 0, 5:10, :])
            nc.scalar.dma_start(out=wq_sb[0][:, 1, :, :], in_=wt_ext[0][:, 1, :, :])
            nc.scalar.dma_start(out=wq_sb[0][:, 2, :, :], in_=wt_ext[0][:, 2, :, :])
            nc.scalar.dma_start(out=wq_sb[0][:, 3, :, :], in_=wt_ext[0][:, 3, :, :])
            nc.scalar.dma_start(out=wq_sb[1][:, 0:2], in_=wt_ext[1][:, 0:2])
            nc.scalar.dma_start(out=wq_sb[1][:, 2:4], in_=wt_ext[1][:, 2:4])
            nc.scalar.dma_start(out=wq_sb[2], in_=wt_ext[2])
            nc.scalar.dma_start(out=wq_sb[3], in_=wt_ext[3])

            def wt_slice(j, jo, k):
                return wq_sb[jo][:, j, k, :]

            # ---- PE warm-up: throwaway matmuls on memset data so the HAM
            # clock gate releases while the first input DMAs are in flight;
            # sized to end roughly when pad0/weights land (~3us) ----
            warm_lhs = singles.tile([128, 1], BF16)
            nc.vector.memset(warm_lhs, 1.0)
            warm_rhs = singles.tile([128, 512], BF16)
            nc.vector.memset(warm_rhs, 0.5)
            warm_ps = wps.tile([1, 512], F32)
            N_WARM = 5
            for i in range(N_WARM):
                nc.tensor.matmul(
                    out=warm_ps,
                    lhsT=warm_lhs,
                    rhs=warm_rhs,
                    start=(i == 0),
                    stop=(i == N_WARM - 1),
                )

            eps_sb = singles.tile([128, 1], F32)
            nc.vector.memset(eps_sb, EPS_EFF)

            # ---- zero-padded modulated input (bf16), border-only memsets ----
            pad_sb = []
            for j in range(NCI):
                p = pads.tile([128, HWPAD, HWPAD], BF16, tag=f"pad{j}")
                nc.gpsimd.memset(p[:, 0, :], 0.0)
                nc.gpsimd.memset(p[:, HWPAD - 1, :], 0.0)
                nc.gpsimd.memset(p[:, 1 : HWPAD - 1, 0], 0.0)
                nc.gpsimd.memset(p[:, 1 : HWPAD - 1, HWPAD - 1], 0.0)
                pad_sb.append(p)

            def mod(j, r0, r1):
                nc.vector.tensor_scalar(
                    out=pad_sb[j][:, 1 + r0 : 1 + r1, 1 : W + 1],
                    in0=xt_sb[j][:, r0:r1, :],
                    scalar1=yb_sb[:, j : j + 1],
                    scalar2=None,
                    op0=mybir.AluOpType.mult,
                )

            mod(0, 0, XSPLIT)
            # ys^2 in bf16 for the demod matmuls; only needs yb
            ys2_sb = singles.tile([128, NCI], BF16)
            nc.vector.tensor_mul(ys2_sb, yb_sb[:, 0:NCI], yb_sb[:, 0:NCI])
            mod(0, XSPLIT, H)
            for j in range(1, NCI):
                mod(j, 0, H)

            rs_sb = singles.tile([128, NCO], F32)

            def conv_mms(jo, half):
                ps = cps.tile([128, 512], F32, tag="ps")
                h0 = half * 16
                idx = 0
                for j in range(NCI):
                    for k in range(KK):
                        kh, kw = divmod(k, 3)
                        rhs = pad_sb[j][:, kh + h0 : kh + h0 + 16, kw : kw + W]
                        nc.tensor.matmul(
                            out=ps,
                            lhsT=wt_slice(j, jo, k),
                            rhs=rhs,
                            start=(idx == 0),
                            stop=(idx == KK * NCI - 1),
                        )
                        idx += 1
                return ps

            def demod_pair(ps, q0):
                # xs2[:, q] for q in (q0, q0+1), then rs = 1/sqrt(xs2+eps)
                for q in (q0, q0 + 1):
                    for j in range(NCI):
                        nc.tensor.matmul(
                            out=ps[:, q - q0 : q - q0 + 1],
                            lhsT=wt_slice(j, q, KK),
                            rhs=ys2_sb[:, j : j + 1],
                            start=(j == 0),
                            stop=(j == NCI - 1),
                        )

            def demod_finish(ps, q0):
                nc.scalar.activation(
                    out=rs_sb[:, q0 : q0 + 2], in_=ps, func=AF.Sqrt, bias=eps_sb
                )
                nc.vector.reciprocal(
                    out=rs_sb[:, q0 : q0 + 2], in_=rs_sb[:, q0 : q0 + 2]
                )

            def epilogue(ps, out_ap, jo, c0, c1):
                # out_ap = conv/rs + bias for columns [c0:c1) of this psum
                nc.scalar.activation(
                    out=out_ap,
                    in_=ps[:, c0:c1],
                    func=AF.Identity,
                    bias=yb_sb[:, NCI + jo : NCI + jo + 1],
                    scale=rs_sb[:, jo : jo + 1],
                )

            xs01_ps = d01.tile([128, 2], F32)
            xs23_ps = d23.tile([128, 2], F32)

            # ---- emission order IS dataflow order under Tile ----
            # PE stream: conv jo0 -> demod01 -> conv jo1 -> demod23 ->
            # conv jo2, jo3 (continuous). ACT stream: weight-DMA triggers,
            # sqrt01, epilogues jo0, sqrt23, epilogues jo1..jo3.
            ps00 = conv_mms(0, 0)
            ps01 = conv_mms(0, 1)
            demod_pair(xs01_ps, 0)
            demod_finish(xs01_ps, 0)
            ot0 = outs.tile([128, 2 * 512], F32, tag="ot")
            epilogue(ps00, ot0[:, 0:512], 0, 0, 512)
            epilogue(ps01, ot0[:, 512:1024], 0, 0, 512)
            nc.sync.dma_start(out=out_ext[0], in_=ot0)

            ps10 = conv_mms(1, 0)
            ps11 = conv_mms(1, 1)
            demod_pair(xs23_ps, 2)
            demod_finish(xs23_ps, 2)
            ot1 = outs.tile([128, 2 * 512], F32, tag="ot")
            epilogue(ps10, ot1[:, 0:512], 1, 0, 512)
            epilogue(ps11, ot1[:, 512:1024], 1, 0, 512)
            nc.sync.dma_start(out=out_ext[1], in_=ot1)

            ps20 = conv_mms(2, 0)
            ps21 = conv_mms(2, 1)
            ot2 = outs.tile([128, 2 * 512], F32, tag="ot")
            epilogue(ps20, ot2[:, 0:512], 2, 0, 512)
            epilogue(ps21, ot2[:, 512:1024], 2, 0, 512)
            nc.sync.dma_start(out=out_ext[2], in_=ot2)

            ps30 = conv_mms(3, 0)
            ot3a = outs.tile([128, 512], F32, tag="o3a")
            epilogue(ps30, ot3a, 3, 0, 512)
            nc.sync.dma_start(out=out_ext[3, :, 0:512], in_=ot3a)
            ps31 = conv_mms(3, 1)
            # final tile in two strips so the terminal DMA is small and the
            # second strip's transfer overlaps the first's
            ot3b = outs.tile([128, 256], F32, tag="o3b")
            ot3c = outs.tile([128, 256], F32, tag="o3c")
            epilogue(ps31, ot3b, 3, 0, 256)
            nc.scalar.dma_start(out=out_ext[3, :, 512:768], in_=ot3b)
            epilogue(ps31, ot3c, 3, 256, 512)
            nc.sync.dma_start(out=out_ext[3, :, 768:1024], in_=ot3c)

            # keep the warm-up matmuls live (cheap PSUM read at the end)
            warm_sink = singles.tile([1, 1], F32)
            nc.vector.tensor_copy(out=warm_sink, in_=warm_ps[0:1, 0:1])
    nc.compile()
    return nc


_NC_CACHE = None


def _get_nc():
    global _NC_CACHE
    if _NC_CACHE is None:
        _NC_CACHE = build_nc()
    return _NC_CACHE


def _prep_inputs(x, y_s, weight, bias):
    # [co, ci, kh, kw] -> [k, ci, co]; append w2 = sum_k wt^2 as slot 9;
    # then tile to [jco, ci_p, jci, 10, co_c] bf16 contiguous.
    wt9 = weight.transpose(2, 3, 1, 0).reshape(KK, CI, CO)
    w2 = (wt9.astype(np.float64) ** 2).sum(axis=0).astype(np.float32)
    full = np.concatenate([wt9, w2[None]], axis=0)  # [10, ci, co]
    wtq = np.ascontiguousarray(
        full.reshape(KK + 1, NCI, 128, NCO, 128).transpose(3, 2, 1, 0, 4)
    ).astype(ml_dtypes.bfloat16)
    in_maps = []
    for b in range(B):
        yb = np.empty((128, 2 * NCI), np.float32)
        yb[:, :NCI] = y_s[b].reshape(NCI, 128).T
        yb[:, NCI:] = bias.reshape(NCO, 128).T
        in_maps.append(
            {
                "x": np.ascontiguousarray(x[b].reshape(NCI, 128, H, W)).astype(
                    ml_dtypes.bfloat16
                ),
                "yb": yb,
                "wt": wtq,
            }
        )
    return in_maps


def _install_trace_support():
    """Dev-only: register the axon NTFF profiling hook + disable the
    remote artifact upload so trace=True works in this container."""
    import sys
    import types

    import concourse.bass_utils as bu

    bu.upload_artifacts = lambda tmpdir: "local://" + str(tmpdir)
    if "antenv.axon_hooks" in sys.modules:
        return
    try:
        from trn_agent_boot.trn_boot import _ntff_profile_via_ctypes

        hook = _ntff_profile_via_ctypes("/opt/axon/libaxon_pjrt.so")
    except Exception:
        return
    mod = types.ModuleType("antenv.axon_hooks")
    mod.get_axon_ntff_profile_hook = lambda: hook
    mod.set_axon_ntff_profile_hook = lambda h: None
    sys.modules["antenv.axon_hooks"] = mod


def run(x, y_s, weight, bias, trace=False, tmpdir=None):
    nc = _get_nc()
    if trace:
        _install_trace_support()
    in_maps = _prep_inputs(x, y_s, weight, bias)
    res = run_bass_kernel_spmd(
        nc, in_maps, core_ids=list(range(B)), trace=trace, tmpdir=tmpdir
    )
    out = np.stack(
        [res.results[b]["out"].reshape(CO, H, W) for b in range(B)]
    ).astype(np.float32)
    return out, res


def kernel(x, y_s, weight, bias):
    out, _ = run(
        np.asarray(x, dtype=np.float32),
        np.asarray(y_s, dtype=np.float32),
        np.asarray(weight, dtype=np.float32),
        np.asarray(bias, dtype=np.float32),
    )
    return out
